# revision 2
# baseline (speedup 1.0000x reference)
"""MD-RNN (4-direction 2D GRU) Trainium2 kernel.

Sharding: 8-way data-parallel over batch (B=256 -> 32 per core); each core runs
all 4 directional 2D-GRU scans as anti-diagonal wavefronts, interleaved so the
tensor engine stays busy while other engines run the gate nonlinearities.

Layout ("transposed" / hidden-on-partition):
  - hidden states stored as h^T tiles: (128 partitions = hidden chunk, cells*B free)
  - per cell, psum accumulates gs^T = Wx_aug^T@patch_aug + Uh^T@h_above + Uh2^T@h_left
    (r,z gate chunks include the input projection + bias; the n-chunk input
    projection goes to a separate psum tile since it is not multiplied by r)
  - gate math on vector/scalar engines in the same transposed layout; the new
    h^T tile directly feeds the next diagonal's matmuls (no transposes anywhere).

The patch tensor (im2col of x, plus a constant-ones row for the bias trick) is
built host-side (pure data movement) and streamed per-diagonal from DRAM.
"""

import numpy as np
import ml_dtypes

GRID = 4
N_IMG = 32
S = N_IMG - (GRID - 1)          # 29 patch positions per axis
B_FULL = 256
N_CORES = 8
B = B_FULL // N_CORES           # 32 batch per core
H = 256
H3 = 3 * H                      # 768
OUT_DIM = 10
K_IN = GRID * GRID + 1          # 16 patch elems + ones row (bias trick)

FWD = list(range(S))                 # 29 entries
BWD = list(range(S - 2, -1, -1))     # 28 entries (reference off-by-one kept)
DIRS = [(FWD, FWD), (BWD, FWD), (FWD, BWD), (BWD, BWD)]

CELLS_PER_CHUNK = 16            # 16 cells * B=32 = 512 = one psum bank (fp32)

# Recurrence matmul/storage dtype: "bf16" or "f32" (f32 storage + float32r matmuls)
RD_MODE = "bf16"
GX_F32R = False                  # input-projection matmul as float32r (full rate)


def _diag_infos():
    """Per direction: list over diagonals of (i_lo, i_hi, global cell base)."""
    infos = []
    base = 0
    for (yi, xi) in DIRS:
        ny, nx = len(yi), len(xi)
        diags = []
        for d in range(ny + nx - 1):
            ilo = max(0, d - (nx - 1))
            ihi = min(d, ny - 1)
            diags.append((ilo, ihi, base))
            base += ihi - ilo + 1
        infos.append(diags)
    return infos, base


DIAG_INFOS, TOT_CELLS = _diag_infos()


def _scan_index_arrays():
    """Image-space (y, x) of every cell in pt order (dir-major, diag-major)."""
    ys, xs = [], []
    for a, (yi, xi) in enumerate(DIRS):
        ny, nx = len(yi), len(xi)
        for d, (ilo, ihi, _) in enumerate(DIAG_INFOS[a]):
            for i in range(ilo, ihi + 1):
                ys.append(yi[i])
                xs.append(xi[d - i])
    return np.asarray(ys), np.asarray(xs)


YS, XS = _scan_index_arrays()


def _chunk_sizes(k):
    nch = (k + CELLS_PER_CHUNK - 1) // CELLS_PER_CHUNK
    lo = k // nch
    rem = k - lo * nch
    return [lo + 1] * rem + [lo] * (nch - rem)


def make_pt(xc):
    """(B, 32, 32) core batch slice -> (17, TOT_CELLS*B) float32 patch matrix."""
    from numpy.lib.stride_tricks import sliding_window_view
    w = sliding_window_view(xc, (GRID, GRID), axis=(1, 2))   # (B, 29, 29, 4, 4)
    p = w[:, YS, XS].reshape(xc.shape[0], TOT_CELLS, GRID * GRID)  # (B, T, 16)
    p = np.ascontiguousarray(p.transpose(2, 1, 0)).reshape(GRID * GRID, -1)
    ones = np.ones((1, p.shape[1]), np.float32)
    pt = np.concatenate([p, ones], axis=0)
    return np.ascontiguousarray(_np_rd(pt))


def _np_rd(x):
    return x.astype(ml_dtypes.bfloat16) if RD_MODE == "bf16" else x.astype(np.float32)


def make_weight_maps(Wx, Uh, Uh2, b, W_out, b_out):
    Wx, Uh, Uh2 = (np.asarray(t, np.float32) for t in (Wx, Uh, Uh2))
    b, W_out, b_out = (np.asarray(t, np.float32) for t in (b, W_out, b_out))
    uh = np.empty((4, 2, 128, 2 * H3), np.float32)
    for a in range(4):
        for kc in range(2):
            uh[a, kc, :, :H3] = Uh[a][kc * 128:(kc + 1) * 128]
            uh[a, kc, :, H3:] = Uh2[a][kc * 128:(kc + 1) * 128]
    wxa = np.empty((4, K_IN, H3), np.float32)
    for a in range(4):
        wxa[a, :GRID * GRID] = Wx[a]
        wxa[a, GRID * GRID] = b[a]
    wo = np.ascontiguousarray(W_out.reshape(8, 128, OUT_DIM))
    bo = np.ascontiguousarray(b_out.reshape(1, OUT_DIM))
    return {
        "uh": _np_rd(uh),
        "wxa": _np_rd(wxa),
        "wo": wo,
        "bo": bo,
    }


def _build_nc():
    import concourse.bacc as bacc
    import concourse.mybir as mybir
    import concourse.tile as tile

    f32 = mybir.dt.float32
    f32r = mybir.dt.float32r
    RD = mybir.dt.bfloat16 if RD_MODE == "bf16" else f32
    AF = mybir.ActivationFunctionType
    ALU = mybir.AluOpType

    nc = bacc.Bacc("TRN2", target_bir_lowering=False, debug=False,
                   num_devices=N_CORES)
    pt_d = nc.dram_tensor("pt", [K_IN, TOT_CELLS * B], RD, kind="ExternalInput")
    uh_d = nc.dram_tensor("uh", [4, 2, 128, 2 * H3], RD, kind="ExternalInput")
    wxa_d = nc.dram_tensor("wxa", [4, K_IN, H3], RD, kind="ExternalInput")
    wo_d = nc.dram_tensor("wo", [8, 128, OUT_DIM], f32, kind="ExternalInput")
    bo_d = nc.dram_tensor("bo", [1, OUT_DIM], f32, kind="ExternalInput")
    out_d = nc.dram_tensor("out", [B, OUT_DIM], f32, kind="ExternalOutput")

    with tile.TileContext(nc) as tc:
        from contextlib import ExitStack
        with ExitStack() as ctx:
            const = ctx.enter_context(tc.tile_pool(name="const", bufs=1))
            ptp = ctx.enter_context(tc.tile_pool(name="ptp", bufs=4))
            ps = ctx.enter_context(tc.tile_pool(name="ps", bufs=7, space="PSUM"))
            psl = ctx.enter_context(tc.tile_pool(name="psl", bufs=1, space="PSUM"))
            hps = [ctx.enter_context(tc.tile_pool(name=f"h{a}", bufs=3))
                   for a in range(4)]
            ew = ctx.enter_context(tc.tile_pool(name="ew", bufs=3))
            hd = ctx.enter_context(tc.tile_pool(name="hd", bufs=1))

            # --- resident weights ---
            uh_sb = {}
            for a in range(4):
                for kc in range(2):
                    t = const.tile([128, 2 * H3], RD, tag=f"uh{a}{kc}")
                    nc.sync.dma_start(out=t, in_=uh_d[a, kc])
                    uh_sb[a, kc] = t
            wxa_sb = {}
            for a in range(4):
                t = const.tile([K_IN, H3], RD, tag=f"wxa{a}")
                nc.sync.dma_start(out=t, in_=wxa_d[a])
                wxa_sb[a] = t
            wo_sb = const.tile([128, 8 * OUT_DIM], f32, tag="wo")
            for c in range(8):
                nc.sync.dma_start(out=wo_sb[:, c * OUT_DIM:(c + 1) * OUT_DIM],
                                  in_=wo_d[c])
            bo_sb = const.tile([1, OUT_DIM], f32, tag="bo")
            nc.sync.dma_start(out=bo_sb, in_=bo_d[:, :])
            ones_sb = const.tile([1, B], f32, tag="ones")
            nc.vector.memset(ones_sb, 1.0)
            zero_h = const.tile([128, 2, 2 * B], RD, tag="zeroh")
            nc.vector.memset(zero_h, 0.0)

            def emit_chunk(a, prev_t, s_a, cbase, c0, c1, ht):
                fd = (c1 - c0) * B
                ptt = ptp.tile([K_IN, CELLS_PER_CHUNK * B], RD, tag="pt")
                nc.sync.dma_start(
                    out=ptt[:, :fd],
                    in_=pt_d[:, (cbase + c0) * B:(cbase + c1) * B])
                above = {kc: prev_t[:, kc, (s_a + c0) * B:(s_a + c1) * B]
                         for kc in (0, 1)}
                left = {kc: prev_t[:, kc, (s_a + 1 + c0) * B:(s_a + 1 + c1) * B]
                        for kc in (0, 1)}

                gate = [None] * 6
                xnb = [None] * 2
                for mc in range(6):
                    pst = ps.tile([128, CELLS_PER_CHUNK * B], f32, tag="g")
                    po = pst[:, :fd]
                    wx_l = wxa_sb[a][:, mc * 128:(mc + 1) * 128]
                    pt_r = ptt[:, :fd]
                    if GX_F32R:
                        wx_l = wx_l.bitcast(f32r)
                        pt_r = pt_r.bitcast(f32r)
                    uh_mm = []
                    for kc in (0, 1):
                        lu = uh_sb[a, kc][:, mc * 128:(mc + 1) * 128]
                        lu2 = uh_sb[a, kc][:, H3 + mc * 128:H3 + (mc + 1) * 128]
                        uh_mm.append((lu, above[kc]))
                        uh_mm.append((lu2, left[kc]))
                    if mc < 4:
                        nc.tensor.matmul(po, wx_l, pt_r, start=True, stop=False)
                        for q, (lhs, rhs) in enumerate(uh_mm):
                            nc.tensor.matmul(po, lhs, rhs, start=False,
                                             stop=(q == 3))
                        gate[mc] = pst
                    else:
                        for q, (lhs, rhs) in enumerate(uh_mm):
                            nc.tensor.matmul(po, lhs, rhs, start=(q == 0),
                                             stop=(q == 3))
                        gate[mc] = pst
                        xt = ps.tile([128, CELLS_PER_CHUNK * B], f32, tag="g")
                        nc.tensor.matmul(xt[:, :fd], wx_l, pt_r,
                                         start=True, stop=True)
                        xnb[mc - 4] = xt

                for kc in (0, 1):
                    rt = ew.tile([128, CELLS_PER_CHUNK * B], RD, tag="r")
                    nc.scalar.activation(rt[:, :fd], gate[kc][:, :fd], AF.Sigmoid)
                    zt = ew.tile([128, CELLS_PER_CHUNK * B], RD, tag="z")
                    nc.scalar.activation(zt[:, :fd], gate[2 + kc][:, :fd],
                                         AF.Sigmoid)
                    xn = ew.tile([128, CELLS_PER_CHUNK * B], RD, tag="xn")
                    nc.scalar.copy(xn[:, :fd], xnb[kc][:, :fd])
                    t1 = ew.tile([128, CELLS_PER_CHUNK * B], RD, tag="t1")
                    nc.vector.tensor_mul(t1[:, :fd], rt[:, :fd],
                                         gate[4 + kc][:, :fd])
                    t2 = ew.tile([128, CELLS_PER_CHUNK * B], RD, tag="t2")
                    nc.vector.tensor_add(t2[:, :fd], t1[:, :fd], xn[:, :fd])
                    nt = ew.tile([128, CELLS_PER_CHUNK * B], RD, tag="n")
                    nc.scalar.activation(nt[:, :fd], t2[:, :fd], AF.Tanh)
                    st = ew.tile([128, CELLS_PER_CHUNK * B], RD, tag="s")
                    nc.vector.tensor_add(st[:, :fd], above[kc], left[kc])
                    dt_ = ew.tile([128, CELLS_PER_CHUNK * B], RD, tag="d")
                    nc.vector.scalar_tensor_tensor(
                        dt_[:, :fd], st[:, :fd], 0.5, nt[:, :fd],
                        ALU.mult, ALU.subtract)
                    et = ew.tile([128, CELLS_PER_CHUNK * B], RD, tag="e")
                    nc.vector.tensor_mul(et[:, :fd], zt[:, :fd], dt_[:, :fd])
                    nc.vector.tensor_add(ht[:, kc, (1 + c0) * B:(1 + c1) * B],
                                         et[:, :fd], nt[:, :fd])

            # --- main wavefront, 4 directions interleaved per diagonal ---
            h_prev = {a: None for a in range(4)}
            max_nd = max(len(di) for di in DIAG_INFOS)
            for d in range(max_nd):
                for a in range(4):
                    if d >= len(DIAG_INFOS[a]):
                        continue
                    ilo, ihi, cbase = DIAG_INFOS[a][d]
                    k = ihi - ilo + 1
                    ht = hps[a].tile([128, 2, (k + 2) * B], RD, tag=f"h{a}")
                    nc.gpsimd.memset(ht[:, :, 0:B], 0.0)
                    nc.gpsimd.memset(ht[:, :, (k + 1) * B:(k + 2) * B], 0.0)
                    if d == 0:
                        prev_t, k_prev, ilo_prev = zero_h, 0, 0
                    else:
                        prev_t, k_prev, ilo_prev = h_prev[a]
                    s_a = ilo - ilo_prev
                    assert 0 <= s_a and s_a + k <= k_prev + 2, (a, d)
                    c0 = 0
                    for cs in _chunk_sizes(k):
                        emit_chunk(a, prev_t, s_a, cbase, c0, c0 + cs, ht)
                        c0 += cs
                    h_prev[a] = (ht, k, ilo)

            # --- head: logits = hcat @ W_out + b_out ; log_softmax ---
            hfin = []
            for a in range(4):
                ht, k, _ = h_prev[a]
                assert k == 1
                for kc in (0, 1):
                    t = hd.tile([128, B], f32, tag=f"hf{a}{kc}")
                    nc.scalar.copy(t, ht[:, kc, B:2 * B])
                    hfin.append(t)
            pl = psl.tile([B, OUT_DIM], f32, tag="pl")
            for c, t in enumerate(hfin):
                nc.tensor.matmul(pl, t, wo_sb[:, c * OUT_DIM:(c + 1) * OUT_DIM],
                                 start=(c == 0), stop=False)
            nc.tensor.matmul(pl, ones_sb[:1, :B], bo_sb, start=False, stop=True)
            mx = hd.tile([B, 1], f32, tag="mx")
            nc.vector.reduce_max(mx, pl, axis=mybir.AxisListType.X)
            nmx = hd.tile([B, 1], f32, tag="nmx")
            nc.vector.tensor_scalar_mul(nmx, mx, -1.0)
            exv = hd.tile([B, OUT_DIM], f32, tag="exv")
            nc.scalar.activation(exv, pl, AF.Exp, bias=nmx, scale=1.0)
            sm = hd.tile([B, 1], f32, tag="sm")
            nc.vector.reduce_sum(sm, exv, axis=mybir.AxisListType.X)
            lnz = hd.tile([B, 1], f32, tag="lnz")
            nc.scalar.activation(lnz, sm, AF.Ln)
            tot = hd.tile([B, 1], f32, tag="tot")
            nc.vector.tensor_add(tot, lnz, mx)
            ntot = hd.tile([B, 1], f32, tag="ntot")
            nc.vector.tensor_scalar_mul(ntot, tot, -1.0)
            ot = hd.tile([B, OUT_DIM], f32, tag="ot")
            nc.scalar.activation(ot, pl, AF.Identity, bias=ntot, scale=1.0)
            nc.sync.dma_start(out=out_d[:, :], in_=ot)

    nc.compile()
    return nc


_CACHE = {}


def get_nc():
    if "nc" not in _CACHE:
        _CACHE["nc"] = _build_nc()
    return _CACHE["nc"]


def make_in_maps(x, Wx, Uh, Uh2, b, W_out, b_out):
    x = np.asarray(x, np.float32)
    wm = make_weight_maps(Wx, Uh, Uh2, b, W_out, b_out)
    in_maps = []
    for c in range(N_CORES):
        xc = x[c * B:(c + 1) * B]
        m = dict(wm)
        m["pt"] = make_pt(xc)
        in_maps.append(m)
    return in_maps


def kernel(x, Wx, Uh, Uh2, b, W_out, b_out):
    from concourse.bass_utils import run_bass_kernel_spmd
    nc = get_nc()
    in_maps = make_in_maps(x, Wx, Uh, Uh2, b, W_out, b_out)
    res = run_bass_kernel_spmd(nc, in_maps, list(range(N_CORES)))
    out = np.concatenate([res.results[c]["out"] for c in range(N_CORES)], axis=0)
    return out.astype(np.float32)


# revision 3
# speedup vs baseline: 8.0573x; 8.0573x over previous
"""MD-RNN (4-direction 2D GRU) Trainium2 kernel.

Sharding: 8-way data-parallel over batch (B=256 -> 32 per core); each core runs
all 4 directional 2D-GRU scans as anti-diagonal wavefronts, interleaved so the
tensor engine stays busy while other engines run the gate nonlinearities.

Layout ("transposed" / hidden-on-partition):
  - hidden states stored as h^T tiles: (128 partitions = hidden chunk, cells*B free)
  - per cell, psum accumulates gs^T = Wx_aug^T@patch_aug + Uh^T@h_above + Uh2^T@h_left
    (r,z gate chunks include the input projection + bias; the n-chunk input
    projection goes to a separate psum tile since it is not multiplied by r)
  - gate math on vector/scalar engines in the same transposed layout; the new
    h^T tile directly feeds the next diagonal's matmuls (no transposes anywhere).

The patch tensor (im2col of x, plus a constant-ones row for the bias trick) is
built host-side (pure data movement) and streamed per-diagonal from DRAM.
"""

import numpy as np
import ml_dtypes

GRID = 4
N_IMG = 32
S = N_IMG - (GRID - 1)          # 29 patch positions per axis
B_FULL = 256
N_CORES = 8
B = B_FULL // N_CORES           # 32 batch per core
H = 256
H3 = 3 * H                      # 768
OUT_DIM = 10
K_IN = GRID * GRID + 1          # 16 patch elems + ones row (bias trick)

FWD = list(range(S))                 # 29 entries
BWD = list(range(S - 2, -1, -1))     # 28 entries (reference off-by-one kept)
DIRS = [(FWD, FWD), (BWD, FWD), (FWD, BWD), (BWD, BWD)]

CELLS_PER_CHUNK = 16            # 16 cells * B=32 = 512 = one psum bank (fp32)

# Recurrence matmul/storage dtype: "bf16" or "f32" (f32 storage + float32r matmuls)
RD_MODE = "bf16"
GX_F32R = False
REPEAT = 1                      # body repetitions (timing calibration only)                  # input-projection matmul as float32r (full rate)


def _diag_infos():
    """Per direction: list over diagonals of (i_lo, i_hi, global cell base)."""
    infos = []
    base = 0
    for (yi, xi) in DIRS:
        ny, nx = len(yi), len(xi)
        diags = []
        for d in range(ny + nx - 1):
            ilo = max(0, d - (nx - 1))
            ihi = min(d, ny - 1)
            diags.append((ilo, ihi, base))
            base += ihi - ilo + 1
        infos.append(diags)
    return infos, base


DIAG_INFOS, TOT_CELLS = _diag_infos()


def _scan_index_arrays():
    """Image-space (y, x) of every cell in pt order (dir-major, diag-major)."""
    ys, xs = [], []
    for a, (yi, xi) in enumerate(DIRS):
        ny, nx = len(yi), len(xi)
        for d, (ilo, ihi, _) in enumerate(DIAG_INFOS[a]):
            for i in range(ilo, ihi + 1):
                ys.append(yi[i])
                xs.append(xi[d - i])
    return np.asarray(ys), np.asarray(xs)


YS, XS = _scan_index_arrays()


def _chunk_sizes(k):
    nch = (k + CELLS_PER_CHUNK - 1) // CELLS_PER_CHUNK
    lo = k // nch
    rem = k - lo * nch
    return [lo + 1] * rem + [lo] * (nch - rem)


def make_pt(xc):
    """(B, 32, 32) core batch slice -> (17, TOT_CELLS*B) float32 patch matrix."""
    from numpy.lib.stride_tricks import sliding_window_view
    w = sliding_window_view(xc, (GRID, GRID), axis=(1, 2))   # (B, 29, 29, 4, 4)
    p = w[:, YS, XS].reshape(xc.shape[0], TOT_CELLS, GRID * GRID)  # (B, T, 16)
    p = np.ascontiguousarray(p.transpose(2, 1, 0)).reshape(GRID * GRID, -1)
    ones = np.ones((1, p.shape[1]), np.float32)
    pt = np.concatenate([p, ones], axis=0)
    return np.ascontiguousarray(_np_rd(pt))


def _np_rd(x):
    return x.astype(ml_dtypes.bfloat16) if RD_MODE == "bf16" else x.astype(np.float32)


def make_weight_maps(Wx, Uh, Uh2, b, W_out, b_out):
    Wx, Uh, Uh2 = (np.asarray(t, np.float32) for t in (Wx, Uh, Uh2))
    b, W_out, b_out = (np.asarray(t, np.float32) for t in (b, W_out, b_out))
    uh = np.empty((4, 2, 128, 2 * H3), np.float32)
    for a in range(4):
        for kc in range(2):
            uh[a, kc, :, :H3] = Uh[a][kc * 128:(kc + 1) * 128]
            uh[a, kc, :, H3:] = Uh2[a][kc * 128:(kc + 1) * 128]
    wxa = np.empty((4, K_IN, H3), np.float32)
    for a in range(4):
        wxa[a, :GRID * GRID] = Wx[a]
        wxa[a, GRID * GRID] = b[a]
    wo = np.ascontiguousarray(W_out.reshape(8, 128, OUT_DIM))
    bo = np.ascontiguousarray(b_out.reshape(1, OUT_DIM))
    return {
        "uh": _np_rd(uh),
        "wxa": _np_rd(wxa),
        "wo": wo,
        "bo": bo,
    }


def _build_nc():
    import concourse.bacc as bacc
    import concourse.mybir as mybir
    import concourse.tile as tile

    f32 = mybir.dt.float32
    f32r = mybir.dt.float32r
    RD = mybir.dt.bfloat16 if RD_MODE == "bf16" else f32
    AF = mybir.ActivationFunctionType
    ALU = mybir.AluOpType

    nc = bacc.Bacc("TRN2", target_bir_lowering=False, debug=False,
                   num_devices=N_CORES)
    pt_d = nc.dram_tensor("pt", [K_IN, TOT_CELLS * B], RD, kind="ExternalInput")
    uh_d = nc.dram_tensor("uh", [4, 2, 128, 2 * H3], RD, kind="ExternalInput")
    wxa_d = nc.dram_tensor("wxa", [4, K_IN, H3], RD, kind="ExternalInput")
    wo_d = nc.dram_tensor("wo", [8, 128, OUT_DIM], f32, kind="ExternalInput")
    bo_d = nc.dram_tensor("bo", [1, OUT_DIM], f32, kind="ExternalInput")
    out_d = nc.dram_tensor("out", [B, OUT_DIM], f32, kind="ExternalOutput")

    with tile.TileContext(nc) as tc:
        from contextlib import ExitStack
        with ExitStack() as ctx:
            const = ctx.enter_context(tc.tile_pool(name="const", bufs=1))
            ptp = ctx.enter_context(tc.tile_pool(name="ptp", bufs=6))
            ps = ctx.enter_context(tc.tile_pool(name="ps", bufs=7, space="PSUM"))
            psl = ctx.enter_context(tc.tile_pool(name="psl", bufs=1, space="PSUM"))
            hps = [ctx.enter_context(tc.tile_pool(name=f"h{a}", bufs=3))
                   for a in range(4)]
            ew = ctx.enter_context(tc.tile_pool(name="ew", bufs=5))
            hd = ctx.enter_context(tc.tile_pool(name="hd", bufs=1))

            # --- resident weights ---
            uh_sb = {}
            for a in range(4):
                for kc in range(2):
                    t = const.tile([128, 2 * H3], RD, tag=f"uh{a}{kc}")
                    nc.sync.dma_start(out=t, in_=uh_d[a, kc])
                    uh_sb[a, kc] = t
            wxa_sb = {}
            for a in range(4):
                t = const.tile([K_IN, H3], RD, tag=f"wxa{a}")
                nc.sync.dma_start(out=t, in_=wxa_d[a])
                wxa_sb[a] = t
            wo_sb = const.tile([128, 8 * OUT_DIM], f32, tag="wo")
            for c in range(8):
                nc.sync.dma_start(out=wo_sb[:, c * OUT_DIM:(c + 1) * OUT_DIM],
                                  in_=wo_d[c])
            bo_sb = const.tile([1, OUT_DIM], f32, tag="bo")
            nc.sync.dma_start(out=bo_sb, in_=bo_d[:, :])
            ones_sb = const.tile([1, B], f32, tag="ones")
            nc.vector.memset(ones_sb, 1.0)
            zero_h = const.tile([128, 2, 2 * B], RD, tag="zeroh")
            nc.vector.memset(zero_h, 0.0)

            def emit_chunk(a, prev_t, s_a, cbase, c0, c1, ht):
                fd = (c1 - c0) * B
                ptt = ptp.tile([K_IN, CELLS_PER_CHUNK * B], RD, tag="pt")
                nc.sync.dma_start(
                    out=ptt[:, :fd],
                    in_=pt_d[:, (cbase + c0) * B:(cbase + c1) * B])
                above = {kc: prev_t[:, kc, (s_a + c0) * B:(s_a + c1) * B]
                         for kc in (0, 1)}
                left = {kc: prev_t[:, kc, (s_a + 1 + c0) * B:(s_a + 1 + c1) * B]
                        for kc in (0, 1)}

                gate = [None] * 6
                xnb = [None] * 2
                for mc in range(6):
                    pst = ps.tile([128, CELLS_PER_CHUNK * B], f32, tag="g")
                    po = pst[:, :fd]
                    wx_l = wxa_sb[a][:, mc * 128:(mc + 1) * 128]
                    pt_r = ptt[:, :fd]
                    if GX_F32R:
                        wx_l = wx_l.bitcast(f32r)
                        pt_r = pt_r.bitcast(f32r)
                    uh_mm = []
                    for kc in (0, 1):
                        lu = uh_sb[a, kc][:, mc * 128:(mc + 1) * 128]
                        lu2 = uh_sb[a, kc][:, H3 + mc * 128:H3 + (mc + 1) * 128]
                        uh_mm.append((lu, above[kc]))
                        uh_mm.append((lu2, left[kc]))
                    if mc < 4:
                        nc.tensor.matmul(po, wx_l, pt_r, start=True, stop=False)
                        for q, (lhs, rhs) in enumerate(uh_mm):
                            nc.tensor.matmul(po, lhs, rhs, start=False,
                                             stop=(q == 3))
                        gate[mc] = pst
                    else:
                        for q, (lhs, rhs) in enumerate(uh_mm):
                            nc.tensor.matmul(po, lhs, rhs, start=(q == 0),
                                             stop=(q == 3))
                        gate[mc] = pst
                        xt = ps.tile([128, CELLS_PER_CHUNK * B], f32, tag="g")
                        nc.tensor.matmul(xt[:, :fd], wx_l, pt_r,
                                         start=True, stop=True)
                        xnb[mc - 4] = xt

                for kc in (0, 1):
                    rt = ew.tile([128, CELLS_PER_CHUNK * B], RD, tag="r")
                    nc.scalar.activation(rt[:, :fd], gate[kc][:, :fd], AF.Sigmoid)
                    zt = ew.tile([128, CELLS_PER_CHUNK * B], RD, tag="z")
                    nc.scalar.activation(zt[:, :fd], gate[2 + kc][:, :fd],
                                         AF.Sigmoid)
                    xn = ew.tile([128, CELLS_PER_CHUNK * B], RD, tag="xn")
                    nc.scalar.copy(xn[:, :fd], xnb[kc][:, :fd])
                    t1 = ew.tile([128, CELLS_PER_CHUNK * B], RD, tag="t1")
                    nc.vector.tensor_mul(t1[:, :fd], rt[:, :fd],
                                         gate[4 + kc][:, :fd])
                    t2 = ew.tile([128, CELLS_PER_CHUNK * B], RD, tag="t2")
                    nc.vector.tensor_add(t2[:, :fd], t1[:, :fd], xn[:, :fd])
                    nt = ew.tile([128, CELLS_PER_CHUNK * B], RD, tag="n")
                    nc.scalar.activation(nt[:, :fd], t2[:, :fd], AF.Tanh)
                    st = ew.tile([128, CELLS_PER_CHUNK * B], RD, tag="s")
                    nc.vector.tensor_add(st[:, :fd], above[kc], left[kc])
                    dt_ = ew.tile([128, CELLS_PER_CHUNK * B], RD, tag="d")
                    nc.vector.scalar_tensor_tensor(
                        dt_[:, :fd], st[:, :fd], 0.5, nt[:, :fd],
                        ALU.mult, ALU.subtract)
                    et = ew.tile([128, CELLS_PER_CHUNK * B], RD, tag="e")
                    nc.vector.tensor_mul(et[:, :fd], zt[:, :fd], dt_[:, :fd])
                    nc.vector.tensor_add(ht[:, kc, (1 + c0) * B:(1 + c1) * B],
                                         et[:, :fd], nt[:, :fd])

            # --- main wavefront, 4 directions interleaved per diagonal ---
            max_nd = max(len(di) for di in DIAG_INFOS)
            for _rep in range(REPEAT):
              h_prev = {a: None for a in range(4)}
              for d in range(max_nd):
                 for a in range(4):
                    if d >= len(DIAG_INFOS[a]):
                        continue
                    ilo, ihi, cbase = DIAG_INFOS[a][d]
                    k = ihi - ilo + 1
                    ht = hps[a].tile([128, 2, (k + 2) * B], RD, tag=f"h{a}")
                    nc.gpsimd.memset(ht[:, :, 0:B], 0.0)
                    nc.gpsimd.memset(ht[:, :, (k + 1) * B:(k + 2) * B], 0.0)
                    if d == 0:
                        prev_t, k_prev, ilo_prev = zero_h, 0, 0
                    else:
                        prev_t, k_prev, ilo_prev = h_prev[a]
                    s_a = ilo - ilo_prev
                    assert 0 <= s_a and s_a + k <= k_prev + 2, (a, d)
                    c0 = 0
                    for cs in _chunk_sizes(k):
                        emit_chunk(a, prev_t, s_a, cbase, c0, c0 + cs, ht)
                        c0 += cs
                    h_prev[a] = (ht, k, ilo)

            # --- head: logits = hcat @ W_out + b_out ; log_softmax ---
            hfin = []
            for a in range(4):
                ht, k, _ = h_prev[a]
                assert k == 1
                for kc in (0, 1):
                    t = hd.tile([128, B], f32, tag=f"hf{a}{kc}")
                    nc.scalar.copy(t, ht[:, kc, B:2 * B])
                    hfin.append(t)
            pl = psl.tile([B, OUT_DIM], f32, tag="pl")
            for c, t in enumerate(hfin):
                nc.tensor.matmul(pl, t, wo_sb[:, c * OUT_DIM:(c + 1) * OUT_DIM],
                                 start=(c == 0), stop=False)
            nc.tensor.matmul(pl, ones_sb[:1, :B], bo_sb, start=False, stop=True)
            mx = hd.tile([B, 1], f32, tag="mx")
            nc.vector.reduce_max(mx, pl, axis=mybir.AxisListType.X)
            nmx = hd.tile([B, 1], f32, tag="nmx")
            nc.vector.tensor_scalar_mul(nmx, mx, -1.0)
            exv = hd.tile([B, OUT_DIM], f32, tag="exv")
            nc.scalar.activation(exv, pl, AF.Exp, bias=nmx, scale=1.0)
            sm = hd.tile([B, 1], f32, tag="sm")
            nc.vector.reduce_sum(sm, exv, axis=mybir.AxisListType.X)
            lnz = hd.tile([B, 1], f32, tag="lnz")
            nc.scalar.activation(lnz, sm, AF.Ln)
            tot = hd.tile([B, 1], f32, tag="tot")
            nc.vector.tensor_add(tot, lnz, mx)
            ntot = hd.tile([B, 1], f32, tag="ntot")
            nc.vector.tensor_scalar_mul(ntot, tot, -1.0)
            ot = hd.tile([B, OUT_DIM], f32, tag="ot")
            nc.scalar.activation(ot, pl, AF.Identity, bias=ntot, scale=1.0)
            nc.sync.dma_start(out=out_d[:, :], in_=ot)

    nc.compile()
    return nc


_CACHE = {}


def get_nc():
    if "nc" not in _CACHE:
        _CACHE["nc"] = _build_nc()
    return _CACHE["nc"]


def make_in_maps(x, Wx, Uh, Uh2, b, W_out, b_out):
    x = np.asarray(x, np.float32)
    wm = make_weight_maps(Wx, Uh, Uh2, b, W_out, b_out)
    in_maps = []
    for c in range(N_CORES):
        xc = x[c * B:(c + 1) * B]
        m = dict(wm)
        m["pt"] = make_pt(xc)
        in_maps.append(m)
    return in_maps


def kernel(x, Wx, Uh, Uh2, b, W_out, b_out):
    from concourse.bass_utils import run_bass_kernel_spmd
    nc = get_nc()
    in_maps = make_in_maps(x, Wx, Uh, Uh2, b, W_out, b_out)
    res = run_bass_kernel_spmd(nc, in_maps, list(range(N_CORES)))
    out = np.concatenate([res.results[c]["out"] for c in range(N_CORES)], axis=0)
    return out.astype(np.float32)


# revision 4
# speedup vs baseline: 12.6956x; 1.5757x over previous
"""MD-RNN (4-direction 2D GRU) Trainium2 kernel.

Sharding: 8-way data-parallel over batch (B=256 -> 32 per core); each core runs
all 4 directional 2D-GRU scans as anti-diagonal wavefronts, interleaved so the
tensor engine stays busy while other engines run the gate nonlinearities.

Layout ("transposed" / hidden-on-partition):
  - hidden states stored as h^T tiles: (128 partitions = hidden chunk, cells*B free)
  - per cell, psum accumulates gs^T = Wx_aug^T@patch_aug + Uh^T@h_above + Uh2^T@h_left
    (r,z gate chunks include the input projection + bias; the n-chunk input
    projection goes to a separate psum tile since it is not multiplied by r)
  - gate math on vector/scalar engines in the same transposed layout; the new
    h^T tile directly feeds the next diagonal's matmuls (no transposes anywhere).

The patch tensor (im2col of x, plus a constant-ones row for the bias trick) is
built host-side (pure data movement) and streamed per-diagonal from DRAM.
"""

import numpy as np
import ml_dtypes

GRID = 4
N_IMG = 32
S = N_IMG - (GRID - 1)          # 29 patch positions per axis
B_FULL = 256
N_CORES = 8
B = B_FULL // N_CORES           # 32 batch per core
H = 256
H3 = 3 * H                      # 768
OUT_DIM = 10
K_IN = GRID * GRID + 1          # 16 patch elems + ones row (bias trick)

FWD = list(range(S))                 # 29 entries
BWD = list(range(S - 2, -1, -1))     # 28 entries (reference off-by-one kept)
DIRS = [(FWD, FWD), (BWD, FWD), (FWD, BWD), (BWD, BWD)]

CELLS_PER_CHUNK = 16            # 16 cells * B=32 = 512 = one psum bank (fp32)

# Recurrence matmul/storage dtype: "bf16" or "f32" (f32 storage + float32r matmuls)
RD_MODE = "bf16"
GX_F32R = False
REPEAT = 1                      # body repetitions (timing calibration only)                  # input-projection matmul as float32r (full rate)


def _diag_infos():
    """Per direction: list over diagonals of (i_lo, i_hi, global cell base)."""
    infos = []
    base = 0
    for (yi, xi) in DIRS:
        ny, nx = len(yi), len(xi)
        diags = []
        for d in range(ny + nx - 1):
            ilo = max(0, d - (nx - 1))
            ihi = min(d, ny - 1)
            diags.append((ilo, ihi, base))
            base += ihi - ilo + 1
        infos.append(diags)
    return infos, base


DIAG_INFOS, TOT_CELLS = _diag_infos()


def _scan_index_arrays():
    """Image-space (y, x) of every cell in pt order (dir-major, diag-major)."""
    ys, xs = [], []
    for a, (yi, xi) in enumerate(DIRS):
        ny, nx = len(yi), len(xi)
        for d, (ilo, ihi, _) in enumerate(DIAG_INFOS[a]):
            for i in range(ilo, ihi + 1):
                ys.append(yi[i])
                xs.append(xi[d - i])
    return np.asarray(ys), np.asarray(xs)


YS, XS = _scan_index_arrays()


def _chunk_sizes(k):
    nch = (k + CELLS_PER_CHUNK - 1) // CELLS_PER_CHUNK
    lo = k // nch
    rem = k - lo * nch
    return [lo + 1] * rem + [lo] * (nch - rem)


def make_pt(xc):
    """(B, 32, 32) core batch slice -> (17, TOT_CELLS*B) float32 patch matrix."""
    from numpy.lib.stride_tricks import sliding_window_view
    w = sliding_window_view(xc, (GRID, GRID), axis=(1, 2))   # (B, 29, 29, 4, 4)
    p = w[:, YS, XS].reshape(xc.shape[0], TOT_CELLS, GRID * GRID)  # (B, T, 16)
    p = np.ascontiguousarray(p.transpose(2, 1, 0)).reshape(GRID * GRID, -1)
    ones = np.ones((1, p.shape[1]), np.float32)
    pt = np.concatenate([p, ones], axis=0)
    return np.ascontiguousarray(_np_rd(pt))


def _np_rd(x):
    return x.astype(ml_dtypes.bfloat16) if RD_MODE == "bf16" else x.astype(np.float32)


def make_weight_maps(Wx, Uh, Uh2, b, W_out, b_out):
    Wx, Uh, Uh2 = (np.asarray(t, np.float32) for t in (Wx, Uh, Uh2))
    b, W_out, b_out = (np.asarray(t, np.float32) for t in (b, W_out, b_out))
    uh = np.empty((4, 2, 128, 2 * H3), np.float32)
    for a in range(4):
        for kc in range(2):
            uh[a, kc, :, :H3] = Uh[a][kc * 128:(kc + 1) * 128]
            uh[a, kc, :, H3:] = Uh2[a][kc * 128:(kc + 1) * 128]
    wxa = np.empty((4, K_IN, H3), np.float32)
    for a in range(4):
        wxa[a, :GRID * GRID] = Wx[a]
        wxa[a, GRID * GRID] = b[a]
    wo = np.ascontiguousarray(W_out.reshape(8, 128, OUT_DIM))
    bo = np.ascontiguousarray(b_out.reshape(1, OUT_DIM))
    return {
        "uh": _np_rd(uh),
        "wxa": _np_rd(wxa),
        "wo": wo,
        "bo": bo,
    }


def _build_nc():
    import concourse.bacc as bacc
    import concourse.mybir as mybir
    import concourse.tile as tile

    f32 = mybir.dt.float32
    f32r = mybir.dt.float32r
    RD = mybir.dt.bfloat16 if RD_MODE == "bf16" else f32
    AF = mybir.ActivationFunctionType
    ALU = mybir.AluOpType

    nc = bacc.Bacc("TRN2", target_bir_lowering=False, debug=False,
                   num_devices=N_CORES)
    pt_d = nc.dram_tensor("pt", [K_IN, TOT_CELLS * B], RD, kind="ExternalInput")
    uh_d = nc.dram_tensor("uh", [4, 2, 128, 2 * H3], RD, kind="ExternalInput")
    wxa_d = nc.dram_tensor("wxa", [4, K_IN, H3], RD, kind="ExternalInput")
    wo_d = nc.dram_tensor("wo", [8, 128, OUT_DIM], f32, kind="ExternalInput")
    bo_d = nc.dram_tensor("bo", [1, OUT_DIM], f32, kind="ExternalInput")
    out_d = nc.dram_tensor("out", [B, OUT_DIM], f32, kind="ExternalOutput")

    with tile.TileContext(nc) as tc:
        from contextlib import ExitStack
        with ExitStack() as ctx:
            const = ctx.enter_context(tc.tile_pool(name="const", bufs=1))
            ptp = ctx.enter_context(tc.tile_pool(name="ptp", bufs=6))
            ps = ctx.enter_context(tc.tile_pool(name="ps", bufs=8, space="PSUM"))
            hps = [ctx.enter_context(tc.tile_pool(name=f"h{a}", bufs=3))
                   for a in range(4)]
            ew = ctx.enter_context(tc.tile_pool(name="ew", bufs=5))
            hd = ctx.enter_context(tc.tile_pool(name="hd", bufs=1))

            # --- resident weights ---
            uh_sb = {}
            for a in range(4):
                for kc in range(2):
                    t = const.tile([128, 2 * H3], RD, tag=f"uh{a}{kc}")
                    nc.sync.dma_start(out=t, in_=uh_d[a, kc])
                    uh_sb[a, kc] = t
            wxa_sb = {}
            for a in range(4):
                t = const.tile([K_IN, H3], RD, tag=f"wxa{a}")
                nc.sync.dma_start(out=t, in_=wxa_d[a])
                wxa_sb[a] = t
            wo_sb = const.tile([128, 8 * OUT_DIM], f32, tag="wo")
            for c in range(8):
                nc.sync.dma_start(out=wo_sb[:, c * OUT_DIM:(c + 1) * OUT_DIM],
                                  in_=wo_d[c])
            bo_sb = const.tile([1, OUT_DIM], f32, tag="bo")
            nc.sync.dma_start(out=bo_sb, in_=bo_d[:, :])
            ones_sb = const.tile([1, B], f32, tag="ones")
            nc.vector.memset(ones_sb, 1.0)
            zero_h = const.tile([128, 2, 2 * B], RD, tag="zeroh")
            nc.vector.memset(zero_h, 0.0)

            def emit_chunk(a, prev_t, s_a, cbase, c0, c1, ht):
                fd = (c1 - c0) * B
                ptt = ptp.tile([K_IN, CELLS_PER_CHUNK * B], RD, tag="pt")
                nc.sync.dma_start(
                    out=ptt[:, :fd],
                    in_=pt_d[:, (cbase + c0) * B:(cbase + c1) * B])
                above = {kc: prev_t[:, kc, (s_a + c0) * B:(s_a + c1) * B]
                         for kc in (0, 1)}
                left = {kc: prev_t[:, kc, (s_a + 1 + c0) * B:(s_a + 1 + c1) * B]
                        for kc in (0, 1)}

                gate = [None] * 6
                xnb = [None] * 2
                for mc in range(6):
                    pst = ps.tile([128, CELLS_PER_CHUNK * B], f32, tag="g")
                    po = pst[:, :fd]
                    wx_l = wxa_sb[a][:, mc * 128:(mc + 1) * 128]
                    pt_r = ptt[:, :fd]
                    if GX_F32R:
                        wx_l = wx_l.bitcast(f32r)
                        pt_r = pt_r.bitcast(f32r)
                    uh_mm = []
                    for kc in (0, 1):
                        lu = uh_sb[a, kc][:, mc * 128:(mc + 1) * 128]
                        lu2 = uh_sb[a, kc][:, H3 + mc * 128:H3 + (mc + 1) * 128]
                        uh_mm.append((lu, above[kc]))
                        uh_mm.append((lu2, left[kc]))
                    if mc < 4:
                        nc.tensor.matmul(po, wx_l, pt_r, start=True, stop=False)
                        for q, (lhs, rhs) in enumerate(uh_mm):
                            nc.tensor.matmul(po, lhs, rhs, start=False,
                                             stop=(q == 3))
                        gate[mc] = pst
                    else:
                        for q, (lhs, rhs) in enumerate(uh_mm):
                            nc.tensor.matmul(po, lhs, rhs, start=(q == 0),
                                             stop=(q == 3))
                        gate[mc] = pst
                        xt = ps.tile([128, CELLS_PER_CHUNK * B], f32, tag="g")
                        nc.tensor.matmul(xt[:, :fd], wx_l, pt_r,
                                         start=True, stop=True)
                        xnb[mc - 4] = xt

                for kc in (0, 1):
                    rt = ew.tile([128, CELLS_PER_CHUNK * B], RD, tag="r")
                    nc.scalar.activation(rt[:, :fd], gate[kc][:, :fd], AF.Sigmoid)
                    zt = ew.tile([128, CELLS_PER_CHUNK * B], RD, tag="z")
                    nc.scalar.activation(zt[:, :fd], gate[2 + kc][:, :fd],
                                         AF.Sigmoid)
                    xn = ew.tile([128, CELLS_PER_CHUNK * B], RD, tag="xn")
                    nc.scalar.copy(xn[:, :fd], xnb[kc][:, :fd])
                    t1 = ew.tile([128, CELLS_PER_CHUNK * B], RD, tag="t1")
                    nc.vector.tensor_mul(t1[:, :fd], rt[:, :fd],
                                         gate[4 + kc][:, :fd])
                    t2 = ew.tile([128, CELLS_PER_CHUNK * B], RD, tag="t2")
                    nc.vector.tensor_add(t2[:, :fd], t1[:, :fd], xn[:, :fd])
                    nt = ew.tile([128, CELLS_PER_CHUNK * B], RD, tag="n")
                    nc.scalar.activation(nt[:, :fd], t2[:, :fd], AF.Tanh)
                    st = ew.tile([128, CELLS_PER_CHUNK * B], RD, tag="s")
                    nc.gpsimd.tensor_add(st[:, :fd], above[kc], left[kc])
                    dt_ = ew.tile([128, CELLS_PER_CHUNK * B], RD, tag="d")
                    nc.vector.scalar_tensor_tensor(
                        dt_[:, :fd], st[:, :fd], 0.5, nt[:, :fd],
                        ALU.mult, ALU.subtract)
                    et = ew.tile([128, CELLS_PER_CHUNK * B], RD, tag="e")
                    nc.vector.tensor_mul(et[:, :fd], zt[:, :fd], dt_[:, :fd])
                    nc.gpsimd.tensor_add(ht[:, kc, (1 + c0) * B:(1 + c1) * B],
                                         et[:, :fd], nt[:, :fd])

            # --- main wavefront, 4 directions interleaved per diagonal ---
            max_nd = max(len(di) for di in DIAG_INFOS)
            for _rep in range(REPEAT):
              h_prev = {a: None for a in range(4)}
              for d in range(max_nd):
                 for a in range(4):
                    if d >= len(DIAG_INFOS[a]):
                        continue
                    ilo, ihi, cbase = DIAG_INFOS[a][d]
                    k = ihi - ilo + 1
                    ht = hps[a].tile([128, 2, (k + 2) * B], RD, tag=f"h{a}")
                    nc.gpsimd.memset(ht[:, :, 0:B], 0.0)
                    nc.gpsimd.memset(ht[:, :, (k + 1) * B:(k + 2) * B], 0.0)
                    if d == 0:
                        prev_t, k_prev, ilo_prev = zero_h, 0, 0
                    else:
                        prev_t, k_prev, ilo_prev = h_prev[a]
                    s_a = ilo - ilo_prev
                    assert 0 <= s_a and s_a + k <= k_prev + 2, (a, d)
                    c0 = 0
                    for cs in _chunk_sizes(k):
                        emit_chunk(a, prev_t, s_a, cbase, c0, c0 + cs, ht)
                        c0 += cs
                    h_prev[a] = (ht, k, ilo)

            # --- head: logits = hcat @ W_out + b_out ; log_softmax ---
            hfin = []
            for a in range(4):
                ht, k, _ = h_prev[a]
                assert k == 1
                for kc in (0, 1):
                    t = hd.tile([128, B], f32, tag=f"hf{a}{kc}")
                    nc.scalar.copy(t, ht[:, kc, B:2 * B])
                    hfin.append(t)
            pl_t = ps.tile([128, CELLS_PER_CHUNK * B], f32, tag="g")
            pl = pl_t[:B, :OUT_DIM]
            for c, t in enumerate(hfin):
                nc.tensor.matmul(pl, t, wo_sb[:, c * OUT_DIM:(c + 1) * OUT_DIM],
                                 start=(c == 0), stop=False)
            nc.tensor.matmul(pl, ones_sb[:1, :B], bo_sb, start=False, stop=True)
            mx = hd.tile([B, 1], f32, tag="mx")
            nc.vector.reduce_max(mx, pl, axis=mybir.AxisListType.X)
            nmx = hd.tile([B, 1], f32, tag="nmx")
            nc.vector.tensor_scalar_mul(nmx, mx, -1.0)
            exv = hd.tile([B, OUT_DIM], f32, tag="exv")
            nc.scalar.activation(exv, pl, AF.Exp, bias=nmx, scale=1.0)
            sm = hd.tile([B, 1], f32, tag="sm")
            nc.vector.reduce_sum(sm, exv, axis=mybir.AxisListType.X)
            lnz = hd.tile([B, 1], f32, tag="lnz")
            nc.scalar.activation(lnz, sm, AF.Ln)
            tot = hd.tile([B, 1], f32, tag="tot")
            nc.vector.tensor_add(tot, lnz, mx)
            ntot = hd.tile([B, 1], f32, tag="ntot")
            nc.vector.tensor_scalar_mul(ntot, tot, -1.0)
            ot = hd.tile([B, OUT_DIM], f32, tag="ot")
            nc.scalar.activation(ot, pl, AF.Identity, bias=ntot, scale=1.0)
            nc.sync.dma_start(out=out_d[:, :], in_=ot)

    nc.compile()
    return nc


_CACHE = {}


def get_nc():
    if "nc" not in _CACHE:
        _CACHE["nc"] = _build_nc()
    return _CACHE["nc"]


def make_in_maps(x, Wx, Uh, Uh2, b, W_out, b_out):
    x = np.asarray(x, np.float32)
    wm = make_weight_maps(Wx, Uh, Uh2, b, W_out, b_out)
    in_maps = []
    for c in range(N_CORES):
        xc = x[c * B:(c + 1) * B]
        m = dict(wm)
        m["pt"] = make_pt(xc)
        in_maps.append(m)
    return in_maps


def kernel(x, Wx, Uh, Uh2, b, W_out, b_out):
    from concourse.bass_utils import run_bass_kernel_spmd
    nc = get_nc()
    in_maps = make_in_maps(x, Wx, Uh, Uh2, b, W_out, b_out)
    res = run_bass_kernel_spmd(nc, in_maps, list(range(N_CORES)))
    out = np.concatenate([res.results[c]["out"] for c in range(N_CORES)], axis=0)
    return out.astype(np.float32)
